# revision 43
# baseline (speedup 1.0000x reference)
# Trainium2 Bass kernel for nn_CompressedGPT2Attention.
#
# Model: B=2, S=2048, D=1024, H=16 heads of HD=64.
#   qkv = x @ c_attn_w + c_attn_b ; causal attention per head;
#   per-head symmetric projector on the attention output; out = attn @ c_proj_w + b.
#
# Sharding (megatron-style tensor parallel over heads, 8 cores x 2 heads):
#   - every core gets the full hidden_states
#   - c_attn (q,k,v) columns + projectors + c_proj rows are sharded by head
#   - each core computes a full-shape partial of the c_proj output; the
#     all-reduce after c_proj is done on the host (partials are summed there,
#     c_proj bias is also added on the host).
#
# On-core layout strategy: activations are kept feature-major ("transposed",
# features on SBUF partitions) so every matmul contracts over the partition
# dim without ever transposing big intermediates:
#   xT[d, s]   provided by the host (input marshalling), loaded sb-major
#   qT,kT,vT[f,s] = W^T @ xT  (512-col streams; bias via per-partition
#              activation bias during the PSUM drain)
#   v_s[s, hd] = PE transpose of vT, 128x128 tiles (v must be seq-major as
#              the stationary operand of the attn matmul)
#   scoresT[kj, qi] = kT^T-slice matmuls (two heads packed on the PE via
#              tile_position row-tiling, K=64 each); diagonal blocks are
#              column-trimmed to the causally-valid region
#   expT = exp(scoresT/8), one merged ScalarE instruction per kj pair;
#              causal mask via gpsimd affine_select
#   attn_unT[hd, qi] accumulated over kj with lhsT = v; softmax sums
#              ride along as a concurrent ones-column matmul; both are
#              software-pipelined one kj behind the score matmuls so the PE
#              never waits on ScalarE
#   normalization = reciprocal_approx_fast of the matmul-broadcast sums,
#              then one VectorE multiply
#   outT[dout, s] = c_proj partial with the per-head projector pre-folded
#              into the c_proj weights on the host (normalization is
#              per-(head,query) so it commutes with the projector); written
#              back fp32, host sums over cores.
#
# The whole kernel is ONE interleaved stream: after s-block sb's q/k/v
# matmul groups + v transpose, the attention block that becomes ready
# ((qt=sb%4, b=sb//4)) is emitted immediately. This overlaps the qkv
# phase's PE time with attention's ScalarE exp stream, and the qkv groups
# between attention blocks give the normalization + c_proj of block i-1
# free PE time to hide behind (c_proj is injected after the next s-block's
# q group). PSUM budget (8 banks): qkv groups borrow the score pool's
# 2-bank slots, v-transposes borrow the c_proj pool's slots.

import numpy as np

B, S, D, H, HD = 2, 2048, 1024, 16, 64
BS = B * S
N_CORES = 8
HPC = H // N_CORES  # heads per core = 2

_CACHE = {}


def _build(nc):
    import concourse.bass as bass
    import concourse.mybir as mybir
    import concourse.tile as tile
    from contextlib import ExitStack

    f32 = mybir.dt.float32
    bf16 = mybir.dt.bfloat16
    AF = mybir.ActivationFunctionType
    OP = mybir.AluOpType

    # x is marshalled on the host into (sb, kt)-chunk-contiguous layout so
    # each s-block loads with ONE full-bandwidth DMA push (the sync engine's
    # descriptor-push rate and 1KB-line inefficiency otherwise bottleneck
    # the whole stream)
    x_d = nc.dram_tensor(
        "xc", [BS // 512, 128, D // 128, 512], bf16, kind="ExternalInput"
    ).ap()
    # host-marshalled [p, kt, f] so the whole tensor is one full-bandwidth push
    wqkv_d = nc.dram_tensor(
        "w_qkv", [128, D // 128, 3 * HPC * HD], bf16, kind="ExternalInput"
    ).ap()
    b3_d = nc.dram_tensor("b3", [HPC * HD, 3], f32, kind="ExternalInput").ap()
    wcp_d = nc.dram_tensor("w_cp", [HPC * HD, D], bf16, kind="ExternalInput").ap()
    ident_d = nc.dram_tensor("ident", [128, 128], bf16, kind="ExternalInput").ap()
    out_d = nc.dram_tensor("outT", [8, 128, BS], f32, kind="ExternalOutput").ap()

    F = HPC * HD  # 128 features per block (2 heads stacked)
    NB = BS // 512  # 8 s-blocks of 512
    KT = D // 128  # 8 contraction tiles

    with TileCtx(tile, nc) as tc:
        # ---------------- persistent tiles ----------------
        frees = []

        def ptile(shape, dtype, name):
            t, free = tc.tile(shape, dtype, name=name)
            frees.append(free)
            return t

        qT = ptile([128, BS], bf16, "qT")
        kTt = ptile([128, BS], bf16, "kTt")
        vT = ptile([128, BS], bf16, "vT")
        v_s = ptile([128, BS // 128, 128], bf16, "v_s")
        wqkv_sb = ptile([128, KT, 3 * F], bf16, "wqkv_sb")
        wcp_sb = ptile([128, D], bf16, "wcp_sb")
        b3_sb = ptile([128, 3], f32, "b3_sb")
        ones_w = ptile([128, 64], bf16, "ones_w")
        ident = ptile([128, 128], bf16, "ident")
        # one tile per 512-wide s-block so c_proj can start per-block
        cpr = [ptile([128, 512], bf16, f"cpr{i}") for i in range(NB)]

        xT, xT_free = tc.tile([128, KT, BS], bf16, name="xT")

        # ---------------- constants + weights ----------------
        # DMA issue order is tuned so the first q-matmul group (all kt of
        # sb=0) is unblocked as early as possible: per-kt wqkv row-blocks
        # are interleaved with sb=0's x chunks; wcp (first needed at the
        # first c_proj) goes after sb=1.
        nc.any.memset(ones_w[:], 1.0)
        nc.sync.dma_start(ident[:], ident_d)
        nc.sync.dma_start(b3_sb[:], b3_d)

        def load_x(sb):
            # [128, kt, 512] chunk with 8KB contiguous per-partition lines
            nc.sync.dma_start(xT[:, :, sb * 512 : (sb + 1) * 512], x_d[sb])

        nc.sync.dma_start(wqkv_sb[:], wqkv_d)
        load_x(0)
        load_x(1)
        nc.sync.dma_start(wcp_sb[:], wcp_d)
        for sb in range(2, NB):
            load_x(sb)

        with ExitStack() as body:
            # PSUM: sc 2x2 banks + attn 1 + sums 1 + aux 2 = 8 banks
            sc_ps = body.enter_context(tc.tile_pool(name="sc_ps", bufs=2, space="PSUM"))
            attn_ps = body.enter_context(tc.tile_pool(name="attn_ps", bufs=1, space="PSUM"))
            sums_ps = body.enter_context(tc.tile_pool(name="sums_ps", bufs=1, space="PSUM"))
            aux_ps = body.enter_context(tc.tile_pool(name="aux_ps", bufs=2, space="PSUM"))
            epool = body.enter_context(tc.tile_pool(name="epool", bufs=3))
            spool = body.enter_context(tc.tile_pool(name="spool", bufs=2))
            opool = body.enter_context(tc.tile_pool(name="opool", bufs=3))

            def emit_qkv_group(sb, ft):
                # borrows a 2-bank "sc" slot; uses only its first bank
                ps = sc_ps.tile([128, 2, 512], f32, tag="sc", name="qkvg")
                for kt in range(KT):
                    nc.tensor.matmul(
                        ps[:, 0, :],
                        wqkv_sb[:, kt, ft * F : (ft + 1) * F],
                        xT[:, kt, sb * 512 : (sb + 1) * 512],
                        start=(kt == 0),
                        stop=(kt == KT - 1),
                    )
                # drain on VectorE: ScalarE's exp stream is the attention
                # pipeline's co-critical path, keep it clear
                dest = (qT, kTt, vT)[ft]
                nc.vector.tensor_scalar(
                    dest[:, sb * 512 : (sb + 1) * 512], ps[:, 0, :],
                    b3_sb[:, ft : ft + 1], None, OP.add,
                )

            def transpose_sb(sb):
                # v_s[s, hd] tiles for this 512-col chunk via PE transpose
                # (gpsimd cannot read PSUM, so the drain lives on VectorE)
                ps_t = aux_ps.tile([128, 4, 128], bf16, tag="aux", name="ps_t")
                for i in range(4):
                    st = sb * 4 + i
                    nc.tensor.transpose(
                        ps_t[:, i, :], vT[:, st * 128 : (st + 1) * 128], ident[:]
                    )
                nc.vector.tensor_copy(v_s[:, sb * 4 : (sb + 1) * 4, :], ps_t[:])

            def emit_attn_sums(ps_attn, ps_sums, eAB, vs, c0, first, last):
                nc.tensor.matmul(
                    ps_attn[0:64, c0:512], vs[:, 0:64], eAB[:, 0, c0:512],
                    start=first, stop=last, tile_position=(0, 0),
                    skip_group_check=True,
                )
                nc.tensor.matmul(
                    ps_attn[64:128, c0:512], vs[:, 64:128], eAB[:, 1, c0:512],
                    start=first, stop=last, tile_position=(0, 64),
                    skip_group_check=True,
                )
                nc.tensor.matmul(
                    ps_sums[0:64, c0:512], ones_w[:, 0:64], eAB[:, 0, c0:512],
                    start=first, stop=last, tile_position=(0, 0),
                    skip_group_check=True,
                )
                nc.tensor.matmul(
                    ps_sums[64:128, c0:512], ones_w[:, 0:64], eAB[:, 1, c0:512],
                    start=first, stop=last, tile_position=(0, 64),
                    skip_group_check=True,
                )

            def emit_cproj(blk, c0=0, c1=512):
                # drains alternate VectorE/ScalarE so the 2-buf psum rotation
                # is gated by matmul pace, not a single engine's copy chain;
                # output DMAs are merged per dt-pair to halve the push count
                # and alternate sync/scalar rings so transfers overlap
                cl = c1 - c0
                for dp in range(4):
                    ot = opool.tile([128, 2, cl], f32, tag="ot")
                    for half in range(2):
                        dt = 2 * dp + half
                        pcp = aux_ps.tile([128, 512], f32, tag="aux", name="pcp")
                        nc.tensor.matmul(
                            pcp[:, 0:cl], wcp_sb[:, dt * 128 : (dt + 1) * 128],
                            cpr[blk][:, c0:c1], start=True, stop=True,
                        )
                        if half == 0:
                            nc.vector.tensor_copy(ot[:, 0, :], pcp[:, 0:cl])
                        else:
                            nc.scalar.copy(ot[:, 1, :], pcp[:, 0:cl])
                    eng = nc.sync if dp % 2 == 0 else nc.scalar
                    eng.dma_start(
                        out_d[
                            2 * dp : 2 * dp + 2, :,
                            blk * 512 + c0 : blk * 512 + c1,
                        ].rearrange("t p s -> p t s"),
                        ot[:],
                    )

            def emit_attention(qt, b, split_tail=False):
                # split_tail: normalization + c_proj of columns [0,256) are
                # emitted while the last two kj's (which only touch columns
                # [256,512)) are still in flight — used for the final block
                # so its c_proj tail mostly hides under its own attention
                blk = b * 4 + qt
                qi = b * S + qt * 512
                nkj = 4 * (qt + 1)
                ps_attn = attn_ps.tile([128, 512], f32, tag="attn")
                ps_sums = sums_ps.tile([128, 512], f32, tag="sums")
                rec_bc = spool.tile([128, 512], f32, tag="rec_bc")
                pending = None
                for kj in range(nkj):
                    kjc = b * S + kj * 128
                    p = kj - 4 * qt
                    c0 = 128 * p if p > 0 else 0
                    # both heads' scores go into one 2-bank psum tile so
                    # the exp pair is a single ScalarE instruction
                    pscAB = sc_ps.tile([128, 2, 512], f32, tag="sc")
                    nc.tensor.matmul(
                        pscAB[:, 0, c0:512], kTt[0:64, kjc : kjc + 128],
                        qT[0:64, qi + c0 : qi + 512],
                        start=True, stop=True, tile_position=(0, 0),
                    )
                    nc.tensor.matmul(
                        pscAB[:, 1, c0:512], kTt[64:128, kjc : kjc + 128],
                        qT[64:128, qi + c0 : qi + 512],
                        start=True, stop=True, tile_position=(64, 0),
                    )
                    eAB = epool.tile([128, 2, 512], bf16, tag="e")
                    nc.scalar.activation(
                        eAB[:, :, c0:512], pscAB[:, :, c0:512],
                        AF.Exp, scale=0.125,
                    )
                    if p >= 0:
                        # triangle mask on the 128-wide diagonal square
                        for h in range(2):
                            nc.gpsimd.affine_select(
                                eAB[:, h, 128 * p : 128 * (p + 1)],
                                eAB[:, h, 128 * p : 128 * (p + 1)],
                                pattern=[[1, 128]], base=0,
                                channel_multiplier=-1,
                                compare_op=OP.is_ge, fill=0.0,
                            )
                    if pending is not None:
                        emit_attn_sums(*pending)
                        if split_tail and kj == nkj - 2:
                            # columns [0,256) got their last contribution
                            # from kj-1 (p<=1); normalize them now
                            nc.vector.reciprocal_approx_fast(
                                rec_bc[:, 0:256], ps_sums[:, 0:256]
                            )
                            nc.vector.tensor_tensor(
                                cpr[blk][:, 0:256], ps_attn[:, 0:256],
                                rec_bc[:, 0:256], OP.mult,
                            )
                        if split_tail and kj == nkj - 1:
                            emit_cproj(blk, 0, 256)
                    vs = v_s[:, b * 16 + kj, :]
                    pending = (
                        ps_attn, ps_sums, eAB, vs, c0,
                        kj == 0, kj == nkj - 1,
                    )
                emit_attn_sums(*pending)

                # sums are matmul-broadcast across partitions, so one fast
                # DVE reciprocal (which doubles as the PSUM->SBUF move)
                # plus one multiply normalizes the block
                lo = 256 if split_tail else 0
                nc.vector.reciprocal_approx_fast(
                    rec_bc[:, lo:512], ps_sums[:, lo:512]
                )
                nc.vector.tensor_tensor(
                    cpr[blk][:, lo:512], ps_attn[:, lo:512],
                    rec_bc[:, lo:512], OP.mult,
                )
                if split_tail:
                    emit_cproj(blk, 256, 512)
                    return None
                return blk

            prev_blk = None
            for sb in range(NB):
                qt, b = sb % 4, sb // 4
                emit_qkv_group(sb, 0)
                if prev_blk is not None:
                    emit_cproj(prev_blk)
                emit_qkv_group(sb, 1)
                emit_qkv_group(sb, 2)
                transpose_sb(sb)
                prev_blk = emit_attention(qt, b, split_tail=(sb == NB - 1))
            if prev_blk is not None:
                emit_cproj(prev_blk)

        xT_free()
        for free in reversed(frees):
            free()


class TileCtx:
    """Thin helper so _build can use `tc.tile` / `tc.tile_pool` uniformly."""

    def __init__(self, tile_mod, nc):
        self._tc = tile_mod.TileContext(nc)

    def __enter__(self):
        self._tc.__enter__()
        return self._tc

    def __exit__(self, *exc):
        return self._tc.__exit__(*exc)


def _shard_inputs(inputs):
    import ml_dtypes

    bf = ml_dtypes.bfloat16
    # host-side input marshalling: transpose of hidden_states + bf16 rounding
    # for the matmul operands
    xT = np.ascontiguousarray(
        np.asarray(inputs["hidden_states"], dtype=np.float32).reshape(BS, D).T
    ).astype(bf)
    # chunk layout [sb, p, kt, s]: 8KB contiguous per (sb, partition) line
    xc = np.ascontiguousarray(
        xT.reshape(D // 128, 128, BS // 512, 512).transpose(2, 1, 0, 3)
    )
    Wa = np.asarray(inputs["c_attn_w"], dtype=np.float32)
    ba = np.asarray(inputs["c_attn_b"], dtype=np.float32)
    Wp = np.asarray(inputs["c_proj_w"], dtype=np.float32)
    proj = np.asarray(inputs["projectors"], dtype=np.float32)
    ident = np.eye(128, dtype=np.float32).astype(bf)

    in_maps = []
    F = HPC * HD
    for c in range(N_CORES):
        sl = slice(c * F, (c + 1) * F)
        # fold the per-head projector into the c_proj rows for this core:
        # out_rows[h] = proj[h] @ Wcp[rows of head h]  (applied per head)
        wcp_fold = np.empty((F, D), dtype=np.float32)
        for j in range(HPC):
            h = HPC * c + j
            wcp_fold[j * HD : (j + 1) * HD] = (
                proj[h] @ Wp[c * F + j * HD : c * F + (j + 1) * HD, :]
            )
        b3 = np.stack(
            [
                ba[sl],
                ba[D + c * F : D + (c + 1) * F],
                ba[2 * D + c * F : 2 * D + (c + 1) * F],
            ],
            axis=1,
        )
        in_maps.append(
            {
                "xc": xc,
                "w_qkv": np.ascontiguousarray(
                    np.concatenate(
                        [
                            Wa[:, sl],
                            Wa[:, D + c * F : D + (c + 1) * F],
                            Wa[:, 2 * D + c * F : 2 * D + (c + 1) * F],
                        ],
                        axis=1,
                    )
                    .reshape(D // 128, 128, 3 * F)
                    .transpose(1, 0, 2)
                ).astype(bf),
                "b3": np.ascontiguousarray(b3),
                "w_cp": np.ascontiguousarray(wcp_fold).astype(bf),
                "ident": ident,
            }
        )
    return in_maps


def _get_nc():
    if "nc" not in _CACHE:
        from concourse import bacc

        nc = bacc.Bacc("TRN2", debug=False, num_devices=N_CORES)
        _build(nc)
        # Bacc.compile() runs generate_event_semaphores, which spills
        # per-instruction sync waits beyond the single HW wait slot into
        # separate EventSemaphore instructions — without it walrus fails
        # with "Too many sync wait commands".
        nc.compile()
        _CACHE["nc"] = nc
    return _CACHE["nc"]


def _run(inputs, trace=False, trace_kwargs=None):
    from concourse.bass_utils import run_bass_kernel_spmd

    nc = _get_nc()
    in_maps = _shard_inputs(inputs)
    res = run_bass_kernel_spmd(
        nc,
        in_maps,
        core_ids=list(range(N_CORES)),
        trace=trace,
        **(trace_kwargs or {}),
    )
    acc = np.zeros((8, 128, BS), dtype=np.float32)
    for r in res.results:
        acc += np.asarray(r["outT"], dtype=np.float32)
    out = acc.transpose(2, 0, 1).reshape(BS, D)
    out = out + np.asarray(inputs["c_proj_b"], dtype=np.float32)[None, :]
    out = out.reshape(B, S, D)
    return np.ascontiguousarray(out), res


def kernel(**inputs) -> np.ndarray:
    out, _ = _run(inputs, trace=False)
    return out


def simulate_core(inputs, core=0):
    """CoreSim one core's program (for correctness debugging). Returns outT."""
    from concourse.bass_interp import CoreSim

    nc = _get_nc()
    in_maps = _shard_inputs(inputs)
    sim = CoreSim(nc, trace=False)
    for name, arr in in_maps[core].items():
        sim.tensor(name)[:] = arr
    sim.simulate()
    return np.array(sim.tensor("outT"))
